# revision 30
# baseline (speedup 1.0000x reference)
"""Trainium2 Bass kernel for AttentionGuidedMaskStrategy (topk_masking).

Per batch b and side (a->mask_b, b->mask_a):
  v[j]    = sum_i qmask[i] * attn[b, i, j]           (PE, qmask broadcast to
            all 128 lhsT columns -> PSUM holds v replicated on all partitions)
  vt[p,c] = v[c*128+p]                               (4 diag ACT copies + one
            N=4 transpose matmul)
  rank    = #{j : v[j] < vt[p,c]}                    (DVE compare + fused accum)
  mask    = rank + 1 <= 0.3 * n_nonpad_keys          (exact int() truncation)
  out     = mask ? mask_embedding : embed            (copy_predicated)

Data parallel over 8 NeuronCores: 8 batches per core, no collectives.
Layout is p-major (rows 4p..4p+3 on partition p) so every DMA reads one
contiguous run per partition: 128 descriptors instead of 512, cutting the
~4.7ns/descriptor HWDGE issue cost 4x.  1MB attn loads on the sync queue,
512KB embed loads + stores on the gpsimd queue; ACT only does PSUM->SBUF
copies.  Two warmup matmuls keep the PE HAM clock warm into the main loop.
"""

import sys

for _p in ("/opt/trn_rl_repo",):
    if _p not in sys.path:
        sys.path.insert(0, _p)

import numpy as np
from contextlib import ExitStack

from concourse import bacc, bass, mybir
from concourse.bass_utils import run_bass_kernel_spmd
from concourse.tile import TileContext, add_dep_helper

N_CORES = 8
B_LOC = 8      # 64 batches / 8 cores
L = 512        # La == Lb
E = 256
P = 128
NKC = L // P   # 4 chunks of 128
F32 = mybir.dt.float32
U8 = mybir.dt.uint8
F32R = mybir.dt.float32r
OP = mybir.AluOpType

# fp32 matmuls run as two hi/lo weight passes (exact); f32r would be single
# pass but is tf32-precision and flips ~8 top-k rows (rel err 1.9e-2, too
# close to the 2e-2 gate).  bf16 weights would also single-pass it but bass
# rejects mixed bf16-weight/fp32-moving operands.
QM_DT = F32


def _build() -> bass.Bass:
    nc = bacc.Bacc(None, target_bir_lowering=False)

    attn_dt = F32
    attn_a = nc.declare_dram_parameter("attn_a", [B_LOC, L, L], attn_dt, isOutput=False)
    attn_b = nc.declare_dram_parameter("attn_b", [B_LOC, L, L], attn_dt, isOutput=False)
    embed_a = nc.declare_dram_parameter("embed_a", [B_LOC, L, E], F32, isOutput=False)
    embed_b = nc.declare_dram_parameter("embed_b", [B_LOC, L, E], F32, isOutput=False)
    memb = nc.declare_dram_parameter("mask_embedding", [1, E], F32, isOutput=False)
    a_pad = nc.declare_dram_parameter("a_padding_mask", [B_LOC, L], U8, isOutput=False)
    b_pad = nc.declare_dram_parameter("b_padding_mask", [B_LOC, L], U8, isOutput=False)
    out_b = nc.declare_dram_parameter("out_b", [B_LOC, L, E], F32, isOutput=True)
    out_a = nc.declare_dram_parameter("out_a", [B_LOC, L, E], F32, isOutput=True)

    with TileContext(nc) as tc, ExitStack() as ctx:
        const = ctx.enter_context(tc.tile_pool(name="const", bufs=1))
        at_pool = ctx.enter_context(tc.tile_pool(name="at", bufs=8))
        vbc_psum = ctx.enter_context(tc.tile_pool(name="vbc_ps", bufs=3, space="PSUM"))
        rows0_attn = attn_a[0]

        ones_k1 = const.tile([1, P], F32)       # lhsT for partition-broadcast
        nc.vector.memset(ones_k1[:], 1.0)
        ones_k128 = const.tile([P, 1], F32)     # lhsT for partition-sum
        nc.vector.memset(ones_k128[:], 1.0)
        # selector for the v4 -> vt transpose matmul: sel4[p, kc] = (p == 32*kc)
        sel4 = const.tile([P, NKC], F32)
        selio = const.tile([P, NKC], mybir.dt.int32, tag="selio")
        nc.gpsimd.iota(selio[:], pattern=[[-32, NKC]], base=0, channel_multiplier=1)
        nc.vector.tensor_scalar(sel4[:], selio[:], 0, None, op0=OP.is_equal)
        # v4 scratch (double buffered by hand): v chunk kc parked on partition
        # 32*kc; all other partitions stay zero forever
        v4bufs = []
        for i in range(3):
            v4t = const.tile([P, P], F32, tag=f"v4_{i}")
            nc.vector.memset(v4t[:], 0.0)
            v4bufs.append(v4t)

        # selector for the qm_row -> qm transpose: I8[b', b] = (b' == b)
        ident8 = const.tile([B_LOC, B_LOC], F32)
        id8io = const.tile([B_LOC, B_LOC], mybir.dt.int32, tag="id8io")
        nc.gpsimd.iota(id8io[:], pattern=[[-1, B_LOC]], base=0, channel_multiplier=1)
        nc.vector.tensor_scalar(ident8[:], id8io[:], 0, None, op0=OP.is_equal)

        setup_frontier = []
        setup_ctx = ExitStack()

        # HAM warmup: ~3.4us of dummy PE work, gated on the pad-mask DMA so
        # it ends right as the real matmuls become ready - the main loop then
        # starts at the warm 2.4 GHz clock instead of 1.2 GHz.
        wrow = const.tile([1, L], F32, tag="wrow")
        nc.vector.memset(wrow[:], 0.0)

        memb_sb = const.tile([1, E], F32)
        setup_frontier.append(nc.sync.dma_start(out=memb_sb[:], in_=memb[:, :]))

        # padding masks, loaded in natural [b, j] layout (8 fat descriptors,
        # tiny transfers that land early) and transposed on chip
        pad_dmas = {}
        qrow_fs = {}
        for name, ap in (("a", a_pad), ("b", b_pad)):
            pr = const.tile([B_LOC, L], U8, tag=f"pad_{name}")
            pad_dmas[name] = nc.sync.dma_start(out=pr[:], in_=ap[:, :])
            setup_frontier.append(pad_dmas[name])
            qrow_f = const.tile([B_LOC, L], F32, tag=f"qmrow_{name}")
            # qmask = 1.0 where not padded
            setup_frontier.append(
                nc.vector.tensor_scalar(qrow_f[:], pr[:], 0.0, None, op0=OP.is_equal))
            qrow_fs[name] = qrow_f

        # row 0's attn load right after the tiny setup DMAs: it is the longest
        # pole of the pipeline ramp
        at0 = at_pool.tile([P, NKC, L], F32, tag="at")
        at0_dma = nc.sync.dma_start(
            out=at0[:], in_=rows0_attn.rearrange("(p q) j -> p q j", q=NKC))

        # HAM warmup runs while the qmask chain resolves, so the real matmuls
        # start at the warm clock
        wps = vbc_psum.tile([P, L], F32, tag="vbc")
        for wi in range(2):
            wmm = nc.tensor.matmul(wps[:], ones_k1[:], wrow[:],
                                   start=True, stop=True)
            if wi == 0:
                add_dep_helper(wmm.ins, pad_dmas["a"].ins, sync=True,
                               reason="start HAM warmup at pad arrival")

        # qmT_ps[p, q, b] = qm_row[b, 4p+q] via one selector matmul per q
        qm = {}
        qm_ctx = ExitStack()
        qm_psum = qm_ctx.enter_context(
            tc.tile_pool(name="qm_ps", bufs=2, space="PSUM"))
        for name in ("a", "b"):
            qrow_f = qrow_fs[name]
            qt_ps = qm_psum.tile([P, NKC, B_LOC], F32, tag=f"qt_{name}")
            for qq in range(NKC):
                nc.tensor.matmul(
                    qt_ps[:, qq, :],
                    qrow_f[:].rearrange("b (p q) -> b q p", q=NKC)[:, qq],
                    ident8[:], start=True, stop=True)
            q = const.tile([P, B_LOC, NKC], QM_DT, tag=f"qm_{name}")
            setup_frontier.append(
                nc.vector.tensor_copy(
                    q[:], qt_ps[:].rearrange("p q b -> p b q")))
            qm[name] = q
        qm_ctx.close()   # release the transpose psum banks before main pools

        km1_bc = const.tile([P, 2, B_LOC], F32)  # (0.3*len - 1) broadcast down partitions
        memb_bc = const.tile([P, E], F32)        # mask_embedding broadcast down partitions

        def emit_late_setup():

            # PE Matmult fits only ONE sync wait. Setup discipline: every setup
            # matmul gets its own psum bank (tag) and all its operands are
            # produced on DVE, so it waits on at most the monotonic DVE sem.
            # The pool stays open so main-loop psum pools get disjoint banks.
            psum_setup = setup_ctx.enter_context(tc.tile_pool(name="psum_setup", bufs=2,
                                                              space="PSUM"))

            memb_sb2 = const.tile([1, E], F32, tag="memb_sb2")
            nc.vector.tensor_copy(memb_sb2[:], memb_sb[:])

            # per-batch non-padded key counts, per side: sum over partitions
            cnt_ps = psum_setup.tile([1, 2, B_LOC, NKC], F32, tag="setup_ps")
            # row block 0: masks over b keys (k from len_b); block 1: masks over a keys
            nc.tensor.matmul(cnt_ps[:, 0], ones_k128[:], qm["b"][:], start=True, stop=True)
            nc.tensor.matmul(cnt_ps[:, 1], ones_k128[:], qm["a"][:], start=True, stop=True)

            qrow = const.tile([1, 2, B_LOC], F32)
            nc.vector.tensor_reduce(qrow[:], cnt_ps[:], axis=mybir.AxisListType.X, op=OP.add)
            # km1 = 0.3 * count - 1:  mask condition rank < int(q) <=> rank <= q-1.
            # (0.3*count in f32 matches jnp's ratio * count.astype(f32) bit-exactly.)
            nc.vector.tensor_scalar_mul(qrow[:], qrow[:], 0.3)
            km1_row = const.tile([1, 2, B_LOC], F32)
            nc.vector.tensor_scalar_sub(km1_row[:], qrow[:], 1.0)

            # broadcast mask_embedding and km1 down all 128 partitions via
            # ones-outer-product matmuls
            memb_ps = psum_setup.tile([P, E], F32, tag="setup_ps")
            nc.tensor.matmul(memb_ps[:], ones_k1[:], memb_sb2[:], start=True, stop=True)
            setup_frontier.append(nc.vector.tensor_copy(memb_bc[:], memb_ps[:]))

            km1_ps = psum_setup.tile([P, 2 * B_LOC], F32, tag="setup_ps")
            nc.tensor.matmul(km1_ps[:], ones_k1[:],
                             km1_row[:].rearrange("a s b -> a (s b)"),
                             start=True, stop=True)
            setup_frontier.append(
                nc.vector.tensor_copy(km1_bc[:].rearrange("p s b -> p (s b)"), km1_ps[:]))

            # A PE instruction only fits one sync-wait; absorb the setup frontier
            # with a CHAIN of PE drains, one semaphore each (the PE's observed
            # vector clock advances cumulatively), so main-loop matmuls start
            # with a current clock and need at most one wait each.
            for fi in setup_frontier:
                d = nc.tensor.drain()
                add_dep_helper(d.ins, fi.ins, sync=True,
                               reason="absorb setup frontier on PE")
            setup_ctx.close()   # release the 3 setup psum banks for the main loop

        et_pool = ctx.enter_context(tc.tile_pool(name="et", bufs=8))
        vbc_pool = ctx.enter_context(tc.tile_pool(name="vbc", bufs=4))
        scr_pool = ctx.enter_context(tc.tile_pool(name="scr", bufs=4))
        rk_pool = ctx.enter_context(tc.tile_pool(name="rk", bufs=4))
        vt_psum = ctx.enter_context(tc.tile_pool(name="vt_ps", bufs=3, space="PSUM"))

        # (attn, qmask over queries, embed in/out over keys, km1 row-block)
        sides = [
            (attn_a, qm["a"], embed_b, out_b, 0),
            (attn_b, qm["b"], embed_a, out_a, 1),
        ]
        rows = [(b,) + s for b in range(B_LOC) for s in sides]

        def emit_front(r, pending_vt=None):
            """Loads + key sums. vbc[p, j] = sum_i qmask[i] attn[i, j] on every
            partition p (qmask lhsT broadcast to all 128 columns)."""
            b, attn, qmt, emb, outp, si = rows[r]
            if r == 0:
                at = at0
                at_dma = at0_dma
            else:
                at = at_pool.tile([P, NKC, L], F32, tag="at")
                at_dma = nc.sync.dma_start(
                    out=at[:], in_=attn[b].rearrange("(p q) j -> p q j", q=NKC))
            et = et_pool.tile([P, NKC, E], F32, tag="et")
            nc.gpsimd.dma_start(
                out=et[:], in_=emb[b].rearrange("(p q) e -> p q e", q=NKC))

            vbc_ps = vbc_psum.tile([P, L], F32, tag="vbc")
            for ic in range(NKC):
                lhsT = qmt[:, b, ic:ic + 1].to_broadcast([P, P])
                nc.tensor.matmul(vbc_ps[:], lhsT, at[:, ic],
                                 start=(ic == 0), stop=(ic == NKC - 1))
            if pending_vt is not None:
                pending_vt()

            # v4[32*g, m] = v[4*m + g]: each psum partition already holds the
            # full v, so partition 32*g copies its own stride-4 slice
            # (ACT, psum-near engine; single-partition access needs base%32==0)
            v4 = v4bufs[r % 3]
            for g in range(NKC):
                nc.scalar.copy(
                    v4[32 * g:32 * g + 1, :],
                    vbc_ps[32 * g:32 * g + 1, :].rearrange(
                        "a (m q) -> a q m", q=NKC)[:, g])
            # bulk copy v to SBUF so the DVE rank pass runs in 2x mode
            vbc_sb = vbc_pool.tile([P, L], F32, tag="vbc_sb")
            nc.scalar.copy(vbc_sb[:], vbc_ps[:])
            return et, v4, vbc_sb

        def emit_back_pe(r, v4):
            # vt[p, q] = v4[32*q, p] = v[4p+q] via one N=4 selector matmul
            vt_ps = vt_psum.tile([P, NKC], F32, tag="vt")
            nc.tensor.matmul(vt_ps[:], v4[:], sel4[:], start=True, stop=True,
                             skip_group_check=True)
            # tiny hop to SBUF so the DVE rank pass reads no PSUM operand
            vt_sb = rk_pool.tile([P, NKC], F32, tag="vt_sb")
            nc.scalar.copy(vt_sb[:], vt_ps[:])
            return vt_sb

        def emit_back(r, et, vbc_sb, vt_sb):
            b, attn, qmt, emb, outp, si = rows[r]

            # rank[p, kc] = #{j : v[j] < vT[p, kc]}  (the fused
            # compare+accumulate is DVE-only: walrus rejects accum_out on
            # Pool/gpsimd with NCC_IXCG966)
            rank4 = rk_pool.tile([P, NKC], F32, tag="rank")
            for kc in range(NKC):
                scr = scr_pool.tile([P, L], U8, tag="scr")
                nc.vector.tensor_scalar(
                    scr[:], vbc_sb[:], vt_sb[:, kc:kc + 1], None,
                    op0=OP.is_lt, op1=OP.add, accum_out=rank4[:, kc:kc + 1])

            # mask = rank <= q - 1 (integer-valued f32 compare, exact)
            mask4 = rk_pool.tile([P, NKC], U8, tag="mask")
            nc.vector.tensor_scalar(mask4[:], rank4[:], km1_bc[:, si, b:b + 1], None,
                                    op0=OP.is_le)

            # blend in place: et = mask ? mask_embedding : embed, then store
            # (single fused op over all 4 chunks; mask/memb broadcast via
            # stride-0 dims)
            nc.vector.copy_predicated(
                et[:, :, :],
                mask4[:].unsqueeze(2).to_broadcast([P, NKC, E]),
                memb_bc[:].unsqueeze(1).to_broadcast([P, NKC, E]))
            nc.gpsimd.dma_start(
                out=outp[b].rearrange("(p q) e -> p q e", q=NKC), in_=et[:])

        # Software pipeline: row r's back half is emitted after row r+1's
        # sum matmuls so the PE never stalls on the ACT diag copies.
        emit_late_setup()
        prev = None
        for r in range(len(rows)):
            holder = {}
            if prev is not None:
                pr, pet, pv4, pvbc = prev

                def pending_vt(pr=pr, pv4=pv4, holder=holder):
                    holder["vt_sb"] = emit_back_pe(pr, pv4)
            else:
                pending_vt = None
            state = emit_front(r, pending_vt)
            if prev is not None:
                emit_back(pr, pet, pvbc, holder["vt_sb"])
            prev = (r,) + state
        pr, pet, pv4, pvbc = prev
        vt_sb = emit_back_pe(pr, pv4)
        emit_back(pr, pet, pvbc, vt_sb)

    nc.compile()
    return nc


_NC_CACHE = None


def _get_nc() -> bass.Bass:
    global _NC_CACHE
    if _NC_CACHE is None:
        _NC_CACHE = _build()
    return _NC_CACHE


def _run(inputs, trace=False):
    nc = _get_nc()
    in_maps = []
    for c in range(N_CORES):
        sl = slice(c * B_LOC, (c + 1) * B_LOC)
        in_maps.append({
            "attn_a": np.ascontiguousarray(np.asarray(inputs["attn_a"])[sl]),
            "attn_b": np.ascontiguousarray(np.asarray(inputs["attn_b"])[sl]),
            "embed_a": np.ascontiguousarray(np.asarray(inputs["embed_a"])[sl]),
            "embed_b": np.ascontiguousarray(np.asarray(inputs["embed_b"])[sl]),
            "mask_embedding": np.asarray(inputs["mask_embedding"]),
            "a_padding_mask": np.ascontiguousarray(
                np.asarray(inputs["a_padding_mask"])[sl]).view(np.uint8),
            "b_padding_mask": np.ascontiguousarray(
                np.asarray(inputs["b_padding_mask"])[sl]).view(np.uint8),
        })
    res = run_bass_kernel_spmd(nc, in_maps, core_ids=list(range(N_CORES)), trace=trace)
    out_b = np.concatenate([res.results[c]["out_b"] for c in range(N_CORES)], axis=0)
    out_a = np.concatenate([res.results[c]["out_a"] for c in range(N_CORES)], axis=0)
    return (out_b, out_a), res


def kernel(**inputs):
    outs, _ = _run(inputs, trace=False)
    return outs

